# revision 55
# baseline (speedup 1.0000x reference)
"""NT-Xent contrastive loss (SimCLR) on 8 Trainium2 NeuronCores.

Strategy (v3: host-norm + symmetry + fp8 DoubleRow, PE-only dep chains):
  - Host: z = concat(z_i, z_j) [8192, 1024], L2-normalize rows in f32,
    scale by S=16, quantize to fp8 e4m3. The cosine-sim matrix is then
    just G = q @ q.T (scaled by S^2), no on-device normalization.
  - Symmetry: sim is symmetric. In rotated coords (each core's 1024 rows
    at block 0), core c computes column blocks 1-3 in full and blocks 0
    and 4 only as upper supertile-triangles at 512-col granularity (cols
    [512*(m//4), 1024) for m-tile m — keeps every matmul a uniform 512
    columns; ragged widths break the PE weight-load pipelining). Each
    computed entry feeds its row's exp-sum (ACT accum); every block also
    produces column-side exp-sums (per-column sums over the block's
    computed rows, own-supertile columns excluded for blocks 0/4) which
    the host credits to the transposed rows. Every ordered (r,c) pair of
    the full 8192x8192 matrix is covered exactly once across the fleet;
    per-core matmul volume is 4.5 blocks instead of 8.
  - PE runs fp8e4 DoubleRow matmuls (2 k-subtiles of 128 per pass: 256
    deep per 512-col pass at 1 cycle/col = 2x bf16 MAC throughput, the
    TRN2 fp8 peak). Contraction 1024 = 4 DoubleRow groups accumulated in
    PSUM; a dk-major warmup wave over the first three m-tiles starts the
    PE on the first 256 KB DMA slice of chunk 0.
  - Self-diag masked ON PE: an extra [128,128] fp8 matmul diag(-28) x
    diag(+28) = -784*I appended to the accumulation group (G_diag ~ +256,
    so masked logit ~ -529*INVT2 ~ -29.5 -> exp ~ 1.5e-13, negligible).
    This keeps every ACT exp dependent only on the PE semaphore (no DVE
    writes to PSUM), so Tile emits a single merged sem wait per exp and
    the multi-wait splitter leaves ACT alone.
  - Column-side reduction mostly off PE: DVE accumulates sum_m E_m in
    bf16; PE collapses the partition axis with two ones-matmuls per block
    (~2us total; walrus here cannot codegen gpsimd partition_all_reduce
    and Pool's C-axis tensor_reduce is ~100x too slow).
  - Host combines: per-row sumexp = own row-side + column-side chunks
    credited across cores; loss = mean(log(sumexp) - pos/(S^2*T)).

This container's walrus build only accepts ONE semaphore wait per
instruction (and none on CTRL-encoded ones like Drain), while Tile freely
emits several. Three workarounds below: the TileContext epilogue drain's
waits are re-emitted on DVE memsets; a post-pass drops waits on a sem
that only the waiting instruction's own (in-order) engine increments and
whose target count was already reached earlier in that engine's stream;
and a second post-pass splits any remaining multi-wait instruction by
inserting single-wait event-semaphore "carrier" clones just before it on
the same engine stream (the same wait-only encoding Tile itself emits).
Both epilogue all_engine_barriers are load-bearing: without the final
one the NEFF's re-executions (NTFF profiling iterations) start with
dirty semaphore state and run ~15% slower.
"""

import copy

import numpy as np
import ml_dtypes


def _install_tile_drain_patch():
    import concourse.tile as tile
    from concourse import mybir
    from concourse.vector_clock import ScopedClock

    if getattr(tile.TileContext, "_drain_patch_installed", False):
        return

    def _drain_and_barrier(self, tick_clock, wait_clock):
        nc = self.nc
        drain_inst = nc.sync.drain()
        wait_clock.add_sem_waits(
            drain_inst.ins, ScopedClock({None: tick_clock.global_clock})
        )
        waits = list(drain_inst.ins.sync_info.on_wait)
        drain_inst.ins.sync_info.on_wait.clear()

        if waits:
            scr = nc.const_aps.tensor(0.0, (1, 1), mybir.dt.float32)
            for w in waits:
                ms = nc.vector.memset(scr, 0)
                if ms.ins.sync_info is None:
                    ms.ins.sync_info = mybir.SyncInfo(on_wait=[], on_update=[])
                ms.ins.sync_info.on_wait.append(w)

        nc.all_engine_barrier()
        assert self.sems is not None
        popped = nc._tile_sem_poison_stack.pop()
        assert popped is self._sem_poison
        nc.clear_and_free_semaphores(list(self.sems.allocated().values()))
        nc.all_engine_barrier()

    tile.TileContext._drain_and_barrier = _drain_and_barrier
    tile.TileContext._drain_patch_installed = True


_install_tile_drain_patch()

import concourse.bass as bass
import concourse.tile as tile
from concourse import mybir
from concourse.bass_utils import run_bass_kernel_spmd
from concourse.masks import make_identity

P = 128
D = 1024
R = 8192          # 2N rows
MY = 1024         # rows per core (= block size)
NB = 5            # column blocks computed per core (symmetry: 0..4)
KT = 8            # 128-deep k-subtiles in D
DKT = 4           # DoubleRow groups (256-deep each)
MT = 8            # m-tiles per core
CW = 1024         # column chunk width (= one block)
TEMP = 0.07
S = 16.0          # fp8 pre-scale; G = S^2 * sim
DMV = 28.0        # diag-mask matmul operand: adds -DMV^2 to self-sim
INVT2 = float(1.0 / (S * S * TEMP))
FP8 = mybir.dt.float8e4
BF16 = mybir.dt.bfloat16
F32 = mybir.dt.float32
ALU = mybir.AluOpType
ACTF = mybir.ActivationFunctionType
DR = mybir.MatmulPerfMode.DoubleRow

TRACE = False          # set True externally (test harness) for NTFF profiling
LAST_RESULTS = None    # BassKernelResults of the last run (for the harness)

_NC_CACHE = None


_COMPUTE_INSTS = {
    "InstMatmult", "InstLdweights", "InstActivation", "InstTensorTensor",
    "InstTensorScalarPtr", "InstTensorCopy", "InstMemset", "InstTensorReduce",
    "InstTensorScalarAffineSelect",
}


def _elide_self_waits(nc):
    """Drop semaphore waits that are trivially satisfied by same-engine
    program order: a wait on a sem that is only ever incremented by compute
    instructions of the waiting instruction's own engine, with a target
    value already reached by the updates of instructions earlier in that
    (in-order) engine stream."""
    updaters = {}       # sem id -> set of (engine, inst type)
    for f in nc.m.functions:
        for bb in f.blocks:
            for ins in bb.instructions:
                si = getattr(ins, "sync_info", None)
                if si is None:
                    continue
                for u in si.on_update:
                    if u.update_mode != "sem-inc":
                        updaters.setdefault(u.id, set()).add(("!", "!"))
                        continue
                    updaters.setdefault(u.id, set()).add(
                        (ins.engine, type(ins).__name__))
    self_sems = {}      # sem id -> engine (safe to elide for that engine)
    for sid, kinds in updaters.items():
        engines = {e for e, _ in kinds}
        types = {t for _, t in kinds}
        if len(engines) == 1 and types <= _COMPUTE_INSTS:
            self_sems[sid] = next(iter(engines))

    counts = {}         # (engine, sem id) -> completed increments so far
    n = 0
    for f in nc.m.functions:
        for bb in f.blocks:
            for ins in bb.instructions:
                si = getattr(ins, "sync_info", None)
                if si is None:
                    continue
                if si.on_wait:
                    keep = []
                    for w in si.on_wait:
                        eng = self_sems.get(w.id)
                        if (eng == ins.engine
                                and w.wait_mode == "sem-ge-imm"
                                and w.wait_value <= counts.get(
                                    (eng, w.id), 0)):
                            n += 1
                        else:
                            keep.append(w)
                    if len(keep) != len(si.on_wait):
                        del si.on_wait[:]
                        si.on_wait.extend(keep)
                for u in si.on_update:
                    if u.id in self_sems and self_sems[u.id] == ins.engine:
                        counts[(ins.engine, u.id)] = (
                            counts.get((ins.engine, u.id), 0) + u.update_value)
    return n


def _split_multi_waits(nc, templates):
    """Rewrite any instruction carrying >1 sem waits: keep the last wait,
    move each extra onto a fresh single-wait clone of the same-engine no-op
    template inserted immediately before it (engine streams are in-order)."""
    n = 0
    for f in nc.m.functions:
        for bb in f.blocks:
            newlist = []
            for ins in bb.instructions:
                si = getattr(ins, "sync_info", None)
                if si is not None and si.on_wait and len(si.on_wait) > 1:
                    extras = list(si.on_wait[:-1])
                    keep = list(si.on_wait[-1:])
                    tmpl = templates.get(ins.engine)
                    assert tmpl is not None, (
                        f"no wait-carrier template for engine {ins.engine} "
                        f"({type(ins).__name__} {ins.name})"
                    )
                    for w in extras:
                        c = copy.deepcopy(tmpl)
                        c.name = f"wcarrier-{n}"
                        n += 1
                        c.sync_info = mybir.SyncInfo(on_wait=[w], on_update=[])
                        newlist.append(c)
                    del si.on_wait[:]
                    si.on_wait.extend(keep)
                newlist.append(ins)
            bb.instructions[:] = newlist
    return n


def build():
    nc = bass.Bass()
    # [jc][p][ks][col]: element (p, ks, col) of chunk jc = q_rot[jc*CW+col,
    # ks*128+p]; flattened to rows jc*128+p, free ks*CW+col.
    zt = nc.dram_tensor("zt", [NB * P, KT * CW], FP8, kind="ExternalInput")
    # [:, :128] = diag(-DMV), [:, 128:] = diag(+DMV)
    dmask = nc.dram_tensor("dmask", [P, 2 * P], FP8, kind="ExternalInput")
    slots_d = nc.dram_tensor("slots", [P, MT * NB], F32, kind="ExternalOutput")
    pos_d = nc.dram_tensor("pos", [P, MT], F32, kind="ExternalOutput")
    col_d = nc.dram_tensor("colsum", [NB, CW], F32, kind="ExternalOutput")

    templates = {}

    with tile.TileContext(nc) as tc:
        with (
            tc.tile_pool(name="singles", bufs=1) as singles,
            tc.tile_pool(name="epool", bufs=3) as epool,
            tc.tile_pool(name="psum_g", bufs=3, space="PSUM") as psum_g,
            tc.tile_pool(name="psum_c", bufs=1, space="PSUM") as psum_c,
        ):
            zt_sb = [singles.tile([P, KT, CW], FP8, name=f"zt{j}")
                     for j in range(NB)]
            dm_sb = singles.tile([P, 2 * P], FP8)
            I128 = singles.tile([P, P], F32)
            ones_bf = singles.tile([P, 1], BF16)
            slots = singles.tile([P, MT * NB], F32)
            pos = singles.tile([P, MT], F32)
            junk_pos = singles.tile([P, P], F32)
            acc = [singles.tile([P, CW], BF16, name=f"acc{b}")
                   for b in range(NB)]

            colsb = [singles.tile([1, CW], F32, name=f"colsb{b}")
                     for b in range(NB)]
            # wait-carrier scratches (one per engine, never read)
            scr_v = singles.tile([1, 1], F32)
            scr_a = singles.tile([1, 1], F32)
            scr_p = singles.tile([1, 1], F32)

            # --- wait-carrier templates: bare event-semaphore instructions
            # (the same wait-only encoding Tile emits itself; far cheaper
            # than a compute no-op on the busy engines) ---
            c0 = nc.const_aps.tensor(0.0, (1, 1), F32)
            tsem = nc.alloc_semaphore("wcarrier_dummy")
            for eng_name, eng_t in (("vector", mybir.EngineType.DVE),
                                    ("scalar", mybir.EngineType.Activation),
                                    ("gpsimd", mybir.EngineType.Pool),
                                    ("sync", mybir.EngineType.SP),
                                    ("tensor", mybir.EngineType.PE)):
                ins = getattr(nc, eng_name).wait_ge(tsem, 0).ins
                ins.sync_info.on_wait.clear()
                templates[eng_t] = ins

            # DMA launches first: the Pool/SP descriptor generation is the
            # startup critical path. Chunk 0 arrives in dk-granular slices
            # so warmup matmuls can start on the first 256 KB; the other
            # loads go through the idle SP queue in parallel.
            for dk in range(DKT):
                eng = nc.sync if dk < 2 else nc.gpsimd
                eng.dma_start(
                    out=zt_sb[0][:, 2 * dk:2 * dk + 2, :],
                    in_=zt[0:P, 2 * dk * CW:(2 * dk + 2) * CW])
            nc.sync.dma_start(
                out=zt_sb[1][:, :, :], in_=zt[P:2 * P, :])
            nc.gpsimd.dma_start(out=dm_sb[:], in_=dmask[:, :])
            for j in range(2, NB):
                nc.gpsimd.dma_start(
                    out=zt_sb[j][:, :, :],
                    in_=zt[j * P:(j + 1) * P, :])

            make_identity(nc, I128[:, :])
            nc.vector.memset(ones_bf[:], 1.0)
            # blocks 0/4: own-supertile columns get no column-side credit
            nc.vector.memset(acc[0][:, 0:512], 0.0)
            nc.vector.memset(acc[4][:, 0:512], 0.0)
            # warm the ACT Exp table while the input DMAs stream
            nc.scalar.activation(out=scr_a[:], in_=c0, func=ACTF.Exp)

            def tri_geom(jc, m):
                # blocks 0/4: upper supertile-triangle, cols
                # [512*(m//4), 1024) of the block, held at g[:, 0:W]
                coff = 512 * (m // 4) if jc in (0, 4) else 0
                return coff, CW - coff

            def emit_mms(jc, m, g):
                coff, W = tri_geom(jc, m)
                doff = m * P - coff           # diag tile offset within g
                for dk in range(DKT):
                    lhsT = zt_sb[0][:, 2 * dk:2 * dk + 2, m * P:(m + 1) * P]
                    for x in range(0, W, 512):
                        stop = (dk == DKT - 1) and not (
                            jc == 0 and x <= doff < x + 512)
                        nc.tensor.matmul(
                            g[:, x:x + 512],
                            lhsT,
                            zt_sb[jc][:, 2 * dk:2 * dk + 2,
                                      coff + x:coff + x + 512],
                            start=(dk == 0), stop=stop,
                            perf_mode=DR,
                            skip_group_check=True)
                if jc == 0:
                    # self-diag -> -DMV^2 via diag(-DMV).T @ diag(+DMV)
                    nc.tensor.matmul(
                        g[:, doff:doff + P],
                        dm_sb[:, 0:P], dm_sb[:, P:2 * P],
                        start=False, stop=True,
                        skip_group_check=True)

            def emit_post(jc, m, g):
                coff, W = tri_geom(jc, m)
                doff = m * P - coff
                if jc == 4:
                    # positive pair: rotated column = row + 4096 (reads g,
                    # doesn't block the exp)
                    nc.vector.scalar_tensor_tensor(
                        out=junk_pos[:], in0=g[:, doff:doff + P], scalar=1.0,
                        in1=I128[:], op0=ALU.mult, op1=ALU.mult,
                        accum_out=pos[:, m:m + 1])
                e = epool.tile([P, CW], BF16, tag="e")
                nc.scalar.activation(
                    out=e[:, 0:W], in_=g[:, 0:W], func=ACTF.Exp, scale=INVT2,
                    accum_out=slots[:, jc * MT + m:jc * MT + m + 1])
                # column-side partial: acc_b += E_m (bf16, DVE). Triangle
                # blocks credit only columns strictly above their own
                # supertile; m//4 == 1 rows contribute nothing.
                if jc in (0, 4):
                    if m < 4:
                        src = e[:, 512:CW]
                        dst = acc[jc][:, 512:CW]
                        if m == 0:
                            nc.vector.tensor_copy(dst, src)
                        else:
                            nc.vector.tensor_tensor(
                                out=dst, in0=acc[jc][:, 512:CW],
                                in1=src, op=ALU.add)
                else:
                    if m == 0:
                        nc.vector.tensor_copy(acc[jc][:], e[:])
                    else:
                        nc.vector.tensor_tensor(
                            out=acc[jc][:], in0=acc[jc][:],
                            in1=e[:], op=ALU.add)

            def emit_main(jc, m):
                g = psum_g.tile([P, CW], F32, tag="g")
                emit_mms(jc, m, g)
                emit_post(jc, m, g)

            cps = [psum_c.tile([1, 512], F32, name=f"cps{h}")
                   for h in range(CW // 512)]

            def flush_colsum(jc):
                # collapse partition axis: ones-matmul per 512-col half
                for h in range(CW // 512):
                    nc.tensor.matmul(
                        cps[h][0:1, :],
                        ones_bf[:],
                        acc[jc][:, h * 512:(h + 1) * 512],
                        start=True, stop=True,
                        skip_group_check=True)
                    nc.vector.tensor_copy(
                        colsb[jc][:, h * 512:(h + 1) * 512],
                        cps[h][0:1, :])
                nc.gpsimd.dma_start(
                    out=col_d[jc:jc + 1, :], in_=colsb[jc][:])

            # Warmup wave: the first three units of chunk 0 run dk-major so
            # matmuls start as soon as the first dk DMA slice lands instead
            # of waiting for the whole chunk.
            WU = 3
            gs = [psum_g.tile([P, CW], F32, tag="g", name=f"gwu{i}")
                  for i in range(WU)]
            for dk in range(DKT):
                for mu in range(WU):
                    lhsT = zt_sb[0][:, 2 * dk:2 * dk + 2,
                                    mu * P:(mu + 1) * P]
                    for x in range(0, CW, 512):
                        stop = (dk == DKT - 1) and not (x <= mu * P < x + 512)
                        nc.tensor.matmul(
                            gs[mu][:, x:x + 512],
                            lhsT,
                            zt_sb[0][:, 2 * dk:2 * dk + 2, x:x + 512],
                            start=(dk == 0), stop=stop,
                            perf_mode=DR,
                            skip_group_check=True)
            for mu in range(WU):
                nc.tensor.matmul(
                    gs[mu][:, mu * P:mu * P + P],
                    dm_sb[:, 0:P], dm_sb[:, P:2 * P],
                    start=False, stop=True,
                    skip_group_check=True)
                emit_post(0, mu, gs[mu])

            for jc in range(NB):
                for m in range(WU if jc == 0 else 0, MT):
                    emit_main(jc, m)
                    # flush the previous block's column sums one unit late
                    # so the PE isn't parked waiting on ACT
                    if m == (WU if jc == 0 else 0) and jc > 0:
                        flush_colsum(jc - 1)
                    # block 4's column accumulator is complete after m=3
                    # (triangle rows m>=4 contribute nothing) - flush it
                    # early to keep it out of the tail
                    if jc == NB - 1 and m == 4:
                        flush_colsum(NB - 1)
                # stream this block's row-side sums out as soon as its
                # last exp retires (keeps the final-DMA tail short)
                nc.sync.dma_start(
                    out=slots_d[:, jc * MT:(jc + 1) * MT],
                    in_=slots[:, jc * MT:(jc + 1) * MT])

            nc.sync.dma_start(out=pos_d[:, :], in_=pos[:])

    _elide_self_waits(nc)
    _split_multi_waits(nc, templates)
    return nc


def _prep_core_input(q8, c):
    """q8: [8192, 1024] fp8 (normalized*S). Returns the [640, 8192] fp8
    array for core c: rotated rows (own block first), first 5 blocks,
    k-subtile-major layout."""
    zr = np.roll(q8, -c * MY, axis=0)[:NB * MY]          # [5120, 1024]
    # chunk jc: [1024 cols][8 ks][128 p] -> [128 p][8 ks][1024 cols]
    a = zr.reshape(NB, CW, KT, P).transpose(0, 3, 2, 1)  # [5, 128, 8, 1024]
    return np.ascontiguousarray(a.reshape(NB * P, KT * CW))


def kernel(z_i: np.ndarray, z_j: np.ndarray) -> np.ndarray:
    global _NC_CACHE, LAST_RESULTS
    z = np.concatenate([np.asarray(z_i, dtype=np.float32),
                        np.asarray(z_j, dtype=np.float32)], axis=0)
    norm = np.maximum(np.sqrt((z.astype(np.float64) ** 2).sum(axis=1,
                                                              keepdims=True)),
                      1e-8)
    q8 = ((z / norm) * S).astype(ml_dtypes.float8_e4m3)

    dm = np.zeros((P, 2 * P), dtype=ml_dtypes.float8_e4m3)
    idx = np.arange(P)
    dm[idx, idx] = -DMV
    dm[idx, P + idx] = DMV

    in_maps = [{"zt": _prep_core_input(q8, c), "dmask": dm} for c in range(8)]

    if _NC_CACHE is None:
        _NC_CACHE = build()

    res = run_bass_kernel_spmd(
        _NC_CACHE, in_maps, core_ids=list(range(8)), trace=TRACE)
    LAST_RESULTS = res

    sumexp = np.zeros(R, np.float64)
    pos_g = np.zeros(R, np.float64)
    for c in range(8):
        slots = res.results[c]["slots"].astype(np.float64)   # [128, jc*8+m]
        rs = slots.reshape(P, NB, MT).sum(axis=1)            # [p, m]
        sumexp[c * MY:(c + 1) * MY] += rs.T.reshape(MY)      # row i = m*128+p
        posv = res.results[c]["pos"].astype(np.float64)      # [p, m]
        pos_g[c * MY:(c + 1) * MY] = posv.T.reshape(MY) * INVT2
        col = res.results[c]["colsum"].astype(np.float64)    # [5, 1024]
        for b in range(NB):
            gb = (c + b) % 8
            sumexp[gb * MY:(gb + 1) * MY] += col[b]
    loss = np.mean(np.log(sumexp) - pos_g)
    return np.float32(loss)
